# revision 20
# baseline (speedup 1.0000x reference)
"""Trainium2 Bass kernel for nn_DirectedHyperConvLayer (GNN message passing).

Self-contained: accepts FULL inputs, shards across 8 NeuronCores internally,
returns the FULL [50000, 64] float32 output.

Sharding: each core owns a contiguous block of destination rows; the host
routes/sorts edges by destination row into 128-edge tiles grouped by 128-row
destination windows, split into low/high passes so source indices fit
dma_gather's int16 limit. The host also precomputes a combined
[raw_bf16 | normalized_bf16] node table, per-edge destination-hat rows, and
per-tile f8 scatter one-hots. On device, each tile's source rows are fetched
with dma_gather; per-edge cosine weights come from a DVE dot of the gathered
normalized rows against the host-side destination-hat rows; weighted messages
are segment-summed by one-hot matmuls into PSUM per window. An AllGather
exchanges first-hop raw results between the two hops, writing directly into
the raw half of the hop-2 gather table.
"""

import numpy as np
import ml_dtypes
from dataclasses import dataclass

import concourse.bass as bass
import concourse.bacc as bacc
import concourse.mybir as mybir
import concourse.tile as tile

F32 = mybir.dt.float32
BF16 = mybir.dt.bfloat16
F8 = mybir.dt.float8e4
I16 = mybir.dt.int16
NP_F8 = mybir.dt.np(F8)
NP_BF16 = ml_dtypes.bfloat16
P = 128
TB = 8  # tiles per gather call
ALPHA = 0.1
N_AGC = 1  # AllGather chunks
GATHER_PREP = False  # prepare_only + trigger_dma mode


@dataclass
class Config:
    n_nodes: int = 50000
    d: int = 64
    n_cores: int = 8
    rpc: int = 6272  # rows per core (multiple of 128)
    split: int = 32768  # low/high gather split (<= 32768)

    @property
    def nw(self):
        return self.rpc // P

    @property
    def n_pad(self):
        return self.n_cores * self.rpc

    @property
    def nblk(self):
        return self.n_pad // P

    @property
    def ag_chunks(self):
        """list of (w0, nwc, row_base) AllGather chunk descriptors."""
        nw = self.nw
        sizes = [
            nw // N_AGC + (1 if i < nw % N_AGC else 0) for i in range(N_AGC)
        ]
        out = []
        w0 = 0
        base = 0
        for s in sizes:
            out.append((w0, s, base))
            w0 += s
            base += self.n_cores * s * P
        return out

    def m2_remap(self):
        """global padded row id -> row id in the chunk-ordered m2tab."""
        rows = np.arange(self.n_pad, dtype=np.int64)
        k = rows // self.rpc
        rem = rows % self.rpc
        w = rem // P
        p = rem % P
        lut = np.empty(self.n_pad, dtype=np.int64)
        for w0, nwc, base in self.ag_chunks:
            m = (w >= w0) & (w < w0 + nwc)
            lut[m] = base + k[m] * nwc * P + (w[m] - w0) * P + p[m]
        return lut


@dataclass
class StageSched:
    T: np.ndarray  # [2, nw] tiles per (pass, window)
    n_tiles: tuple  # (low, high) tile counts (each % TB == 0)

    @property
    def total_tiles(self):
        return int(self.n_tiles[0] + self.n_tiles[1])

    def tile_windows(self):
        """list over global tile index -> (pass, w, j_in_window, first, last)"""
        out = []
        for p in range(2):
            for w in range(self.T.shape[1]):
                Tw = int(self.T[p, w])
                for j in range(Tw):
                    out.append((p, w, j, j == 0, j == Tw - 1))
        return out


def route_edges(cfg: Config, edge_index, edge_val, hat, r1_map=None):
    """hat: [n_pad, d] float32 normalized embeddings. r1_map optionally
    remaps source row ids into the gather table's row order.
    Returns (sched, per_core list of dicts with idx/val/hatd/s8)."""
    r0 = np.asarray(edge_index[0], dtype=np.int64)
    r1 = np.asarray(edge_index[1], dtype=np.int64)
    if r1_map is not None:
        r1 = r1_map[r1]
    val = np.asarray(edge_val, dtype=np.float32)
    E = r0.shape[0]
    nc_, nw = cfg.n_cores, cfg.nw

    k = r0 // cfg.rpc
    w = (r0 % cfg.rpc) // P
    dloc = r0 % P
    hi = (r1 >= cfg.split).astype(np.int64)
    gid = (k * 2 + hi) * nw + w

    counts = np.bincount(gid, minlength=nc_ * 2 * nw).reshape(nc_, 2, nw)
    T = np.ceil(counts.max(axis=0) / P).astype(np.int64)  # [2, nw]
    for pss in range(2):
        T[pss, nw - 1] += (-int(T[pss].sum())) % TB
    nt_low, nt_high = int(T[0].sum()), int(T[1].sum())
    n_tiles = nt_low + nt_high
    tbase = np.zeros((2, nw), dtype=np.int64)
    tbase[0] = np.cumsum(T[0]) - T[0]
    tbase[1] = nt_low + np.cumsum(T[1]) - T[1]

    order = np.argsort(gid, kind="stable")
    sorted_gid = gid[order]
    starts = np.searchsorted(sorted_gid, np.arange(nc_ * 2 * nw))
    ranks = np.empty(E, dtype=np.int64)
    ranks[order] = np.arange(E) - starts[sorted_gid]

    tile_g = tbase[hi, w] + ranks // P  # global tile per edge
    pos = ranks % P
    idx_val = (r1 - hi * cfg.split).astype(np.int16)
    hat16 = hat.astype(NP_BF16)

    ncalls = n_tiles // TB
    cw = TB * P // 16
    per_core = []
    for kk in range(nc_):
        m = k == kk
        tg, pg = tile_g[m], pos[m]
        slots = tg * P + pg
        idx_flat = np.zeros(n_tiles * P, dtype=np.int16)
        val_flat = np.zeros(n_tiles * P, dtype=np.float32)
        idx_flat[slots] = idx_val[m]
        val_flat[slots] = val[m]
        # idx wrapped: [128, ncalls*cw]
        iw = idx_flat.reshape(ncalls, cw, 16)
        iw = np.transpose(iw, (2, 0, 1)).reshape(16, ncalls * cw)
        idx16 = np.tile(iw, (8, 1))
        # val [128, n_tiles] (partition=slot, free=tile)
        val2d = val_flat.reshape(n_tiles, P).T.copy()
        # hatd [128, n_tiles*64] f8: dest-hat row per edge slot
        hatd_flat = np.zeros((n_tiles * P, cfg.d), dtype=NP_F8)
        hatd_flat[slots] = hat16[r0[m]].astype(NP_F8)
        hatd = (
            hatd_flat.reshape(n_tiles, P, cfg.d)
            .transpose(1, 0, 2)
            .reshape(P, n_tiles * cfg.d)
            .copy()
        )
        # s8 [128, n_tiles*128] f8: scatter one-hot, partition=slot, free=r
        s8 = np.zeros((P, n_tiles * P), dtype=NP_F8)
        s8[pg, tg * P + dloc[m]] = NP_F8(1.0)
        per_core.append({"idx": idx16, "val": val2d, "hatd": hatd, "s8": s8})
    return StageSched(T=T, n_tiles=(nt_low, nt_high)), per_core


def _emit_stage(
    tc, cfg, sched: StageSched, pools, table, idx_t, val_t, hatd_dram, s8_dram,
    out_close,
):
    """Emit one spmm stage. out_close(w, psum_ap, acc_ap, has_low) writes the
    finished window."""
    nc = tc.nc
    nw = cfg.nw
    d = cfg.d

    acc = pools["acc"].tile([P, nw * d], F32, tag="acc")
    nc.vector.memset(acc[:], 0.0)

    tw = sched.tile_windows()
    n_tiles = sched.total_tiles
    assert n_tiles % TB == 0
    ncalls = n_tiles // TB
    nt_low = sched.n_tiles[0]
    cw = TB * P // 16

    win_psum = {}
    for c in range(ncalls):
        pss = 0 if c * TB < nt_low else 1
        tab = table[0 : cfg.split, :] if pss == 0 else table[cfg.split : cfg.n_pad, :]
        t0 = c * TB
        g = pools["g"].tile([P, TB, 2 * d], BF16, tag="g")
        if GATHER_PREP:
            dma_sem = nc.alloc_semaphore(f"gsem_{id(sched)}_{c}")
            nc.gpsimd.dma_gather(
                out_ap=g[:],
                in_ap=tab,
                idxs_ap=idx_t[:, c * cw : (c + 1) * cw],
                num_idxs=TB * P,
                num_idxs_reg=TB * P,
                elem_size=2 * d,
                queue_num=c % 4,
                single_packet=False,
                prepare_only=True,
                sem=dma_sem,
            )
            nc.gpsimd.trigger_dma(count=None, queue_num=c % 4)
        else:
            nc.gpsimd.dma_gather(
                out_ap=g[:],
                in_ap=tab,
                idxs_ap=idx_t[:, c * cw : (c + 1) * cw],
                num_idxs=TB * P,
                num_idxs_reg=TB * P,
                elem_size=2 * d,
                queue_num=c % 4,
                single_packet=False,
            )
        hd = pools["hd"].tile([P, TB, d], F8, tag="hd")
        nc.sync.dma_start(hd[:], hatd_dram[:, t0 * d : (t0 + TB) * d])
        s8t = pools["s8"].tile([P, TB * P], F8, tag="s8")
        nc.sync.dma_start(s8t[:], s8_dram[:, t0 * P : (t0 + TB) * P])

        # cosine dot: prod over gathered hat half
        prod = pools["dve"].tile([P, TB, d], BF16, tag="prod")
        nc.vector.tensor_tensor(
            out=prod[:], in0=hd[:], in1=g[:, :, d : 2 * d],
            op=mybir.AluOpType.mult,
        )
        dot = pools["dve"].tile([P, TB], F32, tag="dot")
        nc.vector.tensor_reduce(
            out=dot[:], in_=prod[:], op=mybir.AluOpType.add,
            axis=mybir.AxisListType.X,
        )
        wv = pools["dve"].tile([P, TB], F32, tag="wv")
        nc.vector.tensor_scalar(
            out=wv[:], in0=dot[:], scalar1=0.5 * ALPHA,
            scalar2=1.0 + 0.5 * ALPHA,
            op0=mybir.AluOpType.mult, op1=mybir.AluOpType.add,
        )
        nc.vector.tensor_tensor(
            out=wv[:], in0=wv[:], in1=val_t[:, t0 : t0 + TB],
            op=mybir.AluOpType.mult,
        )
        msgs = pools["msg"].tile([P, TB, d], BF16, tag="msgs")
        nc.vector.tensor_tensor(
            out=msgs[:], in0=g[:, :, 0:d],
            in1=wv[:].to_broadcast([P, TB, d]),
            op=mybir.AluOpType.mult,
        )
        # scatter matmuls
        for sl in range(TB):
            t = t0 + sl
            pss_t, w, j, first, last = tw[t]
            key = (pss_t, w)
            if key not in win_psum:
                win_psum[key] = pools["pacc"].tile(
                    [P, d], F32, space="PSUM", tag="pacc", name="pacc"
                )
            pw = win_psum[key]
            nc.tensor.matmul(
                out=pw[:],
                lhsT=s8t[:, sl * P : (sl + 1) * P],
                rhs=msgs[:, sl, :],
                start=first,
                stop=last,
            )
            if last:
                if pss_t == 0:
                    nc.scalar.copy(out=acc[:, w * d : (w + 1) * d], in_=pw[:])
                else:
                    has_low = sched.T[0, w] > 0
                    out_close(w, pw[:], acc[:, w * d : (w + 1) * d], has_low)
                del win_psum[key]
    for w in range(nw):
        if sched.T[1, w] == 0:
            out_close(w, None, acc[:, w * d : (w + 1) * d], sched.T[0, w] > 0)


def build_kernel(cfg: Config, sched1: StageSched, sched2: StageSched):
    nc = bacc.Bacc(
        "TRN2",
        target_bir_lowering=False,
        debug=False,
        enable_asserts=False,
        num_devices=cfg.n_cores,
        num_swdge_queues=4,
    )
    d = cfg.d
    tab1 = nc.dram_tensor("tab1", [cfg.n_pad, 2 * d], BF16, kind="ExternalInput")
    phatc = nc.dram_tensor("phatc", [P, cfg.nw * d], BF16, kind="ExternalInput")
    sio = {}
    for s, sch in (("s1", sched1), ("s2", sched2)):
        nt = sch.total_tiles
        sio[s] = {
            "idx": nc.dram_tensor(
                f"{s}_idx", [P, (nt // TB) * (TB * P // 16)], I16,
                kind="ExternalInput",
            ),
            "val": nc.dram_tensor(f"{s}_val", [P, nt], F32, kind="ExternalInput"),
            "hatd": nc.dram_tensor(
                f"{s}_hatd", [P, nt * d], F8, kind="ExternalInput"
            ),
            "s8": nc.dram_tensor(
                f"{s}_s8", [P, nt * P], F8, kind="ExternalInput"
            ),
        }
    out = nc.dram_tensor("out", [cfg.rpc, d], F32, kind="ExternalOutput")

    with tile.TileContext(nc) as tc:
        dram = tc.alloc_tile_pool(name="dram", bufs=1, space="DRAM")
        m2tab = dram.tile([cfg.n_pad, 2 * d], BF16)
        mt_local = dram.tile([cfg.rpc, 2 * d], BF16)

        pools = {
            "g": tc.alloc_tile_pool(name="g", bufs=14),
            "hd": tc.alloc_tile_pool(name="hd", bufs=10),
            "s8": tc.alloc_tile_pool(name="s8p", bufs=10),
            "pacc": tc.alloc_tile_pool(name="pacc", bufs=4, space="PSUM"),
            "dve": tc.alloc_tile_pool(name="dve", bufs=8),
            "msg": tc.alloc_tile_pool(name="msg", bufs=8),
            "acc": tc.alloc_tile_pool(name="accp", bufs=1),
            "io": tc.alloc_tile_pool(name="iop", bufs=1),
            "wout": tc.alloc_tile_pool(name="wout", bufs=3),
        }

        def load_stage_meta(s):
            io = pools["io"]
            t = sio[s]
            nt = t["val"].shape[1]
            idx_t = io.tile([P, t["idx"].shape[1]], I16, tag=f"{s}i", name=f"{s}i")
            nc.sync.dma_start(idx_t[:], t["idx"][:, :])
            val_t = io.tile([P, nt], F32, tag=f"{s}v", name=f"{s}v")
            nc.sync.dma_start(val_t[:], t["val"][:, :])
            return idx_t, val_t, t["hatd"], t["s8"]

        idx1, val1, hatd1, s81 = load_stage_meta("s1")
        idx2, val2, hatd2, s82 = load_stage_meta("s2")
        phat_t = pools["io"].tile([P, cfg.nw * d], BF16, tag="ph", name="ph")
        nc.sync.dma_start(phat_t[:], phatc[:, :])

        # ---------------- stage 1 (tar) ----------------
        chunks = cfg.ag_chunks
        closed = set()
        ag_state = {"next": 0}

        def maybe_fire_ags():
            while ag_state["next"] < N_AGC:
                w0, nwc, base = chunks[ag_state["next"]]
                if not all(w in closed for w in range(w0, w0 + nwc)):
                    break
                nc.gpsimd.collective_compute(
                    "AllGather",
                    mybir.AluOpType.bypass,
                    replica_groups=[list(range(cfg.n_cores))],
                    ins=[mt_local[:][w0 * P : (w0 + nwc) * P, :].opt()],
                    outs=[
                        m2tab[:][
                            base : base + cfg.n_cores * nwc * P, :
                        ].opt()
                    ],
                )
                ag_state["next"] += 1

        def close1(w, psum_ap, acc_ap, has_low):
            ot = pools["wout"].tile([P, 2 * d], BF16, tag="wo1")
            if psum_ap is None:
                nc.scalar.copy(out=ot[:, 0:d], in_=acc_ap)
            elif has_low:
                nc.vector.tensor_tensor(
                    out=ot[:, 0:d], in0=acc_ap, in1=psum_ap,
                    op=mybir.AluOpType.add,
                )
            else:
                nc.scalar.copy(out=ot[:, 0:d], in_=psum_ap)
            nc.scalar.copy(
                out=ot[:, d : 2 * d], in_=phat_t[:, w * d : (w + 1) * d]
            )
            nc.sync.dma_start(
                out=mt_local[:][w * P : (w + 1) * P, :], in_=ot[:]
            )
            closed.add(w)
            maybe_fire_ags()

        _emit_stage(
            tc, cfg, sched1, pools, tab1[:], idx1, val1, hatd1, s81, close1
        )
        assert ag_state["next"] == N_AGC

        # ---------------- stage 2 (src) ----------------
        def close2(w, psum_ap, acc_ap, has_low):
            ot = pools["wout"].tile([P, d], F32, tag="wo2")
            if psum_ap is None:
                nc.scalar.copy(out=ot[:], in_=acc_ap)
            elif has_low:
                nc.vector.tensor_tensor(
                    out=ot[:], in0=acc_ap, in1=psum_ap, op=mybir.AluOpType.add
                )
            else:
                nc.scalar.copy(out=ot[:], in_=psum_ap)
            nc.sync.dma_start(out=out[w * P : (w + 1) * P, :], in_=ot[:])

        _emit_stage(
            tc, cfg, sched2, pools, m2tab[:], idx2, val2, hatd2, s82, close2
        )

        for p in reversed(list(pools.values())):
            p.release()
        dram.release()

    nc.compile()
    return nc


def prepare(cfg: Config, inputs):
    embs = np.asarray(inputs["pois_embs"], dtype=np.float32)
    embp = np.zeros((cfg.n_pad, cfg.d), dtype=np.float32)
    embp[: cfg.n_nodes] = embs
    nrm = np.sqrt((embp * embp).sum(axis=1, keepdims=True))
    hat = embp / np.maximum(nrm, 1e-8)
    tab1 = np.concatenate(
        [embp.astype(NP_BF16), hat.astype(NP_BF16)], axis=1
    )
    hat16 = hat.astype(NP_BF16)

    sched1, meta1 = route_edges(
        cfg, inputs["tar_edge_index"], inputs["tar_edge_val"], hat
    )
    sched2, meta2 = route_edges(
        cfg, inputs["src_edge_index"], inputs["src_edge_val"], hat,
        r1_map=cfg.m2_remap(),
    )
    in_maps = []
    for k in range(cfg.n_cores):
        phatck = (
            hat16[k * cfg.rpc : (k + 1) * cfg.rpc]
            .reshape(cfg.nw, P, cfg.d)
            .transpose(1, 0, 2)
            .reshape(P, cfg.nw * cfg.d)
            .copy()
        )
        in_maps.append(
            {
                "tab1": tab1,
                "phatc": phatck,
                "s1_idx": meta1[k]["idx"], "s1_val": meta1[k]["val"],
                "s1_hatd": meta1[k]["hatd"], "s1_s8": meta1[k]["s8"],
                "s2_idx": meta2[k]["idx"], "s2_val": meta2[k]["val"],
                "s2_hatd": meta2[k]["hatd"], "s2_s8": meta2[k]["s8"],
            }
        )
    return sched1, sched2, in_maps


def assemble_output(cfg: Config, results):
    out = np.zeros((cfg.n_nodes, cfg.d), dtype=np.float32)
    for k, r in enumerate(results):
        lo = k * cfg.rpc
        hi = min(lo + cfg.rpc, cfg.n_nodes)
        if hi > lo:
            out[lo:hi] = r["out"][0 : hi - lo]
    return out


_CACHE = {}


def kernel(**inputs):
    import concourse.bass_utils as bass_utils

    cfg = Config()
    sched1, sched2, in_maps = prepare(cfg, inputs)
    key = (sched1.n_tiles, sched2.n_tiles, tuple(sched1.T.ravel()), tuple(sched2.T.ravel()))
    nc = _CACHE.get(key)
    if nc is None:
        nc = build_kernel(cfg, sched1, sched2)
        _CACHE[key] = nc
    res = bass_utils.run_bass_kernel_spmd(
        nc, in_maps, core_ids=list(range(cfg.n_cores)), trace=False
    )
    out = assemble_output(cfg, res.results)
    return out.astype(np.float32, copy=False)
